# revision 48
# baseline (speedup 1.0000x reference)
"""Trainium2 Bass kernel for per-sample covariance pooling + fc + L2 norm.

Reference computation (per sample b of B=32):
    xc  = x[b] - mean(x[b], axis=0)            # x[b]: [N=20000, D=64]
    cov = xc.T @ xc / (N-1)                    # [64, 64]
    out = normalize(cov.flatten() @ W + b)     # [256]

Kernel formulation (scale/norm invariant; the L2 norm cancels scales):
    G = x.T @ x on device (fp8 x);  R = s s^T/(N(N-1)) + I on HOST (exact)
    feat = 32*(G/(N-1) - R)                    # = 32*(cov - I), fp16
    out = normalize(feat @ W + 32*(b + rowsum_diag(W)))        # identical

x rides fp8 e4m3 (TRN flavor), halving the dominant DMA term; the
identity shift makes the host-folded bias row carry cov's ~1.0
diagonal exactly, and W/feat stay fp16 because this data's cov
residual is ~10x the CLT estimate (fp8 there couples a measured 2e-2
into the output; x-only fp8 costs 2.3e-3). The host computes the tiny
per-sample mean outer product exactly, so the device G pass is a pure
chunked self-matmul: 64-col chunks pair into 128-col blocks, one
FWL matmul per pair (cross blocks discarded, diagonal blocks summed by
DVE). That pegs the PE at its ~53 ns/pair streaming wall -- measured
faster than fp8 DoubleRow (weight-load bound) and 64x64 array tiling
(issue-rate bound). Sharding: data-parallel over batch, 4 samples/core.
"""

import os
import sys

import numpy as np
import ml_dtypes

for _p in ("/opt/trn_rl_repo",):
    if _p not in sys.path:
        sys.path.append(_p)

# Problem shapes (hardcoded per contract).
B, N, D, OUT = 32, 20000, 64, 256
NCORES = 8
BPC = B // NCORES            # samples per core
P = 128                      # SBUF partitions / matmul contraction tile
NCH = 158                    # 128-row chunks after padding (even)
NPR = NCH // 2               # 79 chunk pairs
NPAD = NCH * P               # 20224 rows after zero padding
KC = (D * D) // P            # 32 unfolded fc contraction chunks
KCF = 24                     # fc chunks after the symmetry fold: cov is
                             # symmetric, so rows d>=32, e<32 of the feature
                             # matrix duplicate rows d<32, e>=32; their W
                             # rows are folded together on the host
SC_FEAT = 32.0               # feat scale (cancelled by the L2 norm)
# x DMA schedule per sample: (pair offset, pairs per DMA). Sample 0 ramps
# small-to-large so the PE starts while the DMA rings are still spinning up;
# later samples use few big tiles (the pipe is warm and PE paces the run).
# Only 8 HWDGE DMA contexts exist, assigned round-robin across BOTH rings,
# and the queue engines fair-share packets across every in-flight DMA: DMA
# n's enqueue blocks until DMA n-8 completes. ~320 KB tiles keep the
# 8-deep in-flight window at ~7 us of PE work, so early tiles still land
# roughly in consumption order instead of bunching at the end.
X_TILES0 = [(0, 4), (4, 8), (12, 14), (26, 26), (52, 27)]
X_TILES = [(0, 20), (20, 20), (40, 20), (60, 19)]
X_TILES3 = [(0, 27), (27, 26), (53, 22), (75, 4)]
PREWARM = int(os.environ.get("PREWARM", "20"))
W_SLICES = 8                 # 256 KB fp16 W slices on ring 1
FILL_PER_TILE = int(os.environ.get("FILL", "0"))  # HAM dummy matmuls per x tile
FILL0 = int(os.environ.get("FILL0", "0"))  # same, sample 0 only (DMA ramp)
FILL_SAMPLE_END = int(os.environ.get("FILL_SE", "0"))  # dummies at sample end
G_MODE = os.environ.get("G_MODE", "fwl")  # "fwl" (FWL pairs) | "dr" (DoubleRow)

_CACHE = {}


def _split_drain_and_barrier(self, tick_clock, wait_clock):
    """Replacement for TileContext._drain_and_barrier emitting one drain per
    sem wait: this walrus vintage rejects >1 sync-wait per instruction."""
    import bass_rust
    import concourse.mybir as mybir

    drain_bi = self.nc.sync.drain()
    inst = drain_bi.ins
    wait_clock.add_sem_waits(
        drain_bi.ins, bass_rust.ScopedClock({None: tick_clock.global_clock})
    )
    waits = list(inst.sync_info.on_wait) if inst.sync_info else []
    if len(waits) > 1:
        # one pure sem-wait NoOp per extra wait (cheaper than extra drains)
        inst.sync_info = mybir.SyncInfo(on_wait=waits[:1], on_update=[])
        for w in waits[1:]:
            nop = mybir.InstNoOp(
                name=f"tailwait-{w.ant_name}",
                engine=mybir.EngineType.SP,
                sync_info=mybir.SyncInfo(on_wait=[w], on_update=[]),
                bass_nofuse=True,
            )
            self.nc.sync.add_instruction(nop)

    self.nc.all_engine_barrier()
    assert self.sems is not None
    popped = self.nc._tile_sem_poison_stack.pop()
    assert popped is self._sem_poison
    self.nc.clear_and_free_semaphores(list(self.sems.allocated().values()))
    self.nc.all_engine_barrier()


def _build_nc():
    import types

    import concourse.bass as bass
    import concourse.mybir as mybir
    from concourse.tile import TileContext

    dt = mybir.dt
    AF = mybir.ActivationFunctionType
    DR = mybir.MatmulPerfMode.DoubleRow
    nc = bass.Bass()

    xin = nc.dram_tensor("xin", [BPC, NCH * D * P], dt.float8e4, kind="ExternalInput")
    win = nc.dram_tensor("win", [P, KCF * OUT], dt.float16, kind="ExternalInput")
    # cols 0:OUT: 256*(b + diag-rowsum of W); cols OUT:OUT+BPC: ones (same
    # row -- matmul operands must start at partition 0/32/64)
    bin_ = nc.dram_tensor("bin", [1, OUT + BPC], dt.float16, kind="ExternalInput")
    # rim[d, bb, e] = 32*(s_bb[d] s_bb[e]/(N(N-1)) + I[d,e]) (host, exact)
    rim = nc.dram_tensor("rim", [D, BPC * D], dt.float32, kind="ExternalInput")
    yout = nc.dram_tensor("yout", [BPC, OUT], dt.float32, kind="ExternalOutput")

    # The walrus vintage here supports only ONE sync-wait on data
    # instructions (DMA pseudo ops, TensorCopy, ...). The whole kernel is
    # structured so every emitted instruction needs at most one wait:
    #  - x tiles get one pool slot per DMA (no slot reuse -> 0 waits)
    #  - per-sample psum G tiles are not reused (gpsum bufs=BPC)
    #  - all cross-engine joins are relayed so same-engine waits merge
    #  - PE "observes" the W/bias DMA lanes early via dummy matmuls and
    #    the bias matmul; DVE observes the rim DMA, so the per-sample
    #    feat writes and the fc matmuls only carry one fresh wait each.
    tc = TileContext(nc)
    tc._drain_and_barrier = types.MethodType(_split_drain_and_barrier, tc)
    with tc:
        with (
            tc.tile_pool(name="const", bufs=1) as cpool,
            tc.tile_pool(
                name="xp",
                bufs=len(X_TILES0) + 2 * len(X_TILES) + len(X_TILES3),
            ) as xpool,
            tc.tile_pool(name="small", bufs=2) as spool,
            tc.tile_pool(name="featp", bufs=1) as fpool,
            tc.tile_pool(name="gpsum", bufs=BPC, space="PSUM") as gpool,
            tc.tile_pool(name="opsum", bufs=1, space="PSUM") as opool,
        ):
            # Small constants ride ring 1 (ACT HWDGE) which idles during
            # sample 0; the SWDGE's slow descriptor generation would stall
            # the x stream behind the 64 KB rim transfer otherwise.
            w_sb = cpool.tile([P, KCF * OUT], dt.float16)
            bias_sb = cpool.tile([1, OUT + BPC], dt.float16)
            rim_sb = cpool.tile([D, BPC, D], dt.float32)
            nc.gpsimd.dma_start(out=bias_sb[:], in_=bin_[:])
            nc.gpsimd.dma_start(
                out=rim_sb[:], in_=rim[:].rearrange("p (b f) -> p b f", b=BPC)
            )

            ring = [nc.sync, nc.scalar]
            rr = [0]

            def ring_dma(out, in_, force=None):
                r = force if force is not None else rr[0] % 2
                if force is None:
                    rr[0] += 1
                ring[r].dma_start(out=out, in_=in_)

            WSL = KCF * OUT // W_SLICES
            wq = list(range(W_SLICES))  # pending W slice ids

            def issue_w_slices(k):
                for _ in range(k):
                    if wq:
                        c = wq.pop(0)
                        ring_dma(
                            w_sb[:, c * WSL : (c + 1) * WSL],
                            win[:, c * WSL : (c + 1) * WSL],
                            force=1,
                        )

            # feat_sb[p, c, bb] = flattened 32*(C - I) for sample bb in fc
            # chunk layout: element k = c*128 + p of C.flatten(). Using C's
            # symmetry, k = d*64+e maps to (p = (d%2)*64 + e, c = d//2): no
            # transpose needed. fp16: the cov residual here is ~10x larger
            # than the CLT estimate, so fp8 feat/W couple a 2e-2 error into
            # the output -- measured; only x itself tolerates fp8.
            feat_sb = fpool.tile([P, KCF, BPC], dt.float16)

            po = opool.tile([BPC, OUT], dt.float32)
            pdum = opool.tile([1, 512], dt.float32, tag="pdum")

            # Prewarm bridge: the HAM clock gate needs ~3.4 us of CONTINUOUS
            # PE activity; an idle gap before tile 0 lands resets it and the
            # first sample then runs at half clock. The memset is DVE's
            # FIRST op (no DMA deps, so dummies start at ~0.5 us) and the
            # dummy count is sized to span past tile 0's arrival (~7.5 us)
            # even as the clock ramp shortens each dummy from 426 to 213 ns.
            if PREWARM:
                dumsrc = cpool.tile([P, 512], dt.bfloat16)
                nc.vector.memset(dumsrc[:], 0.5)
                for _ in range(PREWARM):
                    nc.tensor.matmul(
                        pdum[:], lhsT=dumsrc[:, 0:1], rhs=dumsrc[:, 0:512],
                        start=True, stop=True,
                    )

            # DVE observes the rim DMA lane once (after the memset, so the
            # prewarm is not gated on the rim transfer), letting each
            # sample's feat writes carry only their PE wait.
            obs = spool.tile([1, 1], dt.float32, tag="obs")
            nc.vector.tensor_copy(obs[:], rim_sb[0:1, 0:1, 0])

            def do_sample(bb):
                if G_MODE == "dr":
                    pg = gpool.tile([D, D], dt.float32, tag="pg")
                else:
                    pg = gpool.tile([P, P], dt.float32, tag="pg")
                sched = (
                    X_TILES0 if bb == 0 else X_TILES3 if bb == 3 else X_TILES
                )
                for ti, (p0, npr) in enumerate(sched):
                    xt = xpool.tile([P, npr * P], dt.float8e4, tag="xt")
                    # sample 0 serial on ring 0: halving bandwidth across two
                    # rings would delay the first tile (and PE start)
                    ring_dma(
                        xt[:],
                        xin[bb, p0 * P * P : (p0 + npr) * P * P].rearrange(
                            "(p f) -> p f", p=P
                        ),
                        force=0 if bb == 0 else None,
                    )
                    if bb in (1, 2):
                        issue_w_slices(2)
                    for j in range(npr):
                        blk = xt[:, j * P : (j + 1) * P]
                        if G_MODE == "dr":
                            ch = blk.rearrange("p (two f) -> p two f", two=2)
                            nc.tensor.matmul(
                                pg[:], lhsT=ch, rhs=ch,
                                start=(p0 + j == 0), stop=(p0 + j == NPR - 1),
                                perf_mode=DR,
                            )
                        else:
                            nc.tensor.matmul(
                                pg[:], lhsT=blk, rhs=blk,
                                start=(p0 + j == 0), stop=(p0 + j == NPR - 1),
                            )
                    # HAM-warming filler: keeps the PE activity monitor from
                    # dropping the clock during DMA slack. No new deps.
                    for _ in range(FILL0 if bb == 0 else FILL_PER_TILE):
                        nc.tensor.matmul(
                            pdum[:], lhsT=xt[:, 0:1], rhs=xt[:, 0:512],
                            start=True, stop=True,
                        )
                # feat = 32*(G/(N-1) - R) with R host-computed; strided even/
                # odd column writes build the fc chunk layout directly.
                if G_MODE == "dr":
                    gs = pg[:]
                else:
                    # G = even-chunk block + odd-chunk block; DVE reads only
                    # one PSUM operand per op, so stage one block in SBUF.
                    godd = spool.tile([D, D], dt.float32, tag="godd")
                    nc.vector.tensor_copy(godd[:], pg[D:P, D:P])
                    gsum = spool.tile([D, D], dt.float32, tag="gsum")
                    nc.vector.tensor_add(gsum[:], pg[0:D, 0:D], godd[:])
                    gs = gsum[:]
                # feat = 32*(G/(N-1) - s s^T/(N(N-1)) - I), cast to fp16.
                # Chunks 0..15: rows d<32 of the (symmetric) matrix, one
                # row-pair per chunk, read column-wise from gs. Chunks
                # 16..23: the bottom-right 32x32 block, 4 rows per chunk
                # (out partition group i holds rows d = 32+4j+i).
                kf = SC_FEAT / (N - 1.0)
                H = D // 2
                ge = gs[:, 0:H].rearrange("p (c two) -> p c two", two=2)
                re = rim_sb[:, bb, 0:H].rearrange("p (c two) -> p c two", two=2)
                nc.vector.scalar_tensor_tensor(
                    feat_sb[0:D, 0:16, bb], ge[:, :, 0], kf,
                    re[:, :, 0], op0=mybir.AluOpType.mult,
                    op1=mybir.AluOpType.subtract,
                )
                nc.vector.scalar_tensor_tensor(
                    feat_sb[D:P, 0:16, bb], ge[:, :, 1], kf,
                    re[:, :, 1], op0=mybir.AluOpType.mult,
                    op1=mybir.AluOpType.subtract,
                )
                gbr = gs[H:D, H:D].rearrange("p (j four) -> p j four", four=4)
                rbr = rim_sb[H:D, bb, H:D].rearrange(
                    "p (j four) -> p j four", four=4
                )
                for i in range(4):
                    nc.vector.scalar_tensor_tensor(
                        feat_sb[32 * i : 32 * (i + 1), 16:KCF, bb],
                        gbr[:, :, i], kf, rbr[:, :, i],
                        op0=mybir.AluOpType.mult,
                        op1=mybir.AluOpType.subtract,
                    )
                # keep the PE array warm across the sample-boundary stall
                for _ in range(0 if bb == 0 else FILL_SAMPLE_END):
                    nc.tensor.matmul(
                        pdum[:], lhsT=xt[:, 0:1], rhs=xt[:, 0:512],
                        start=True, stop=True,
                    )

            do_sample(0)
            do_sample(1)
            do_sample(2)
            issue_w_slices(W_SLICES)  # any stragglers
            # PE observes every W slice's DMA lane (all land during samples
            # 1-2) and opens the fc accumulation with the bias row HERE, so
            # none of it sits on the post-stream critical tail and the fc
            # matmuls carry no DMA waits of their own.
            for c in range(W_SLICES):
                nc.tensor.matmul(
                    pdum[0:1, 0:1], lhsT=w_sb[0:1, c * WSL : c * WSL + 1],
                    rhs=w_sb[0:1, c * WSL : c * WSL + 1],
                    start=True, stop=True,
                )
            nc.tensor.matmul(
                po[:], lhsT=bias_sb[0:1, OUT : OUT + BPC], rhs=bias_sb[0:1, 0:OUT],
                start=True, stop=False,
            )
            do_sample(3)

            # fc: out[bb, o] = bias'[o] + sum_k feat[k, bb] * W[k, o]
            for c in range(KCF):
                nc.tensor.matmul(
                    po[:],
                    lhsT=feat_sb[:, c, :],
                    rhs=w_sb[:, c * OUT : (c + 1) * OUT],
                    start=False,
                    stop=(c == KCF - 1),
                )

            # L2 normalize rows: out = po / ||po||. ACT fuses square+rowsum
            # in one op; the tiny sqrt stays on ACT (no extra engine hop).
            sq = spool.tile([BPC, OUT], dt.float32, tag="sq")
            ss = spool.tile([BPC, 1], dt.float32, tag="ss")
            nc.scalar.activation(sq[:], po[:], AF.Square, accum_out=ss[:])
            nrm = spool.tile([BPC, 1], dt.float32, tag="nrm")
            nc.scalar.activation(nrm[:], ss[:], AF.Sqrt)
            inv = spool.tile([BPC, 1], dt.float32, tag="inv")
            nc.vector.reciprocal(inv[:], nrm[:])
            out_sb = spool.tile([BPC, OUT], dt.float32, tag="osb")
            nc.vector.tensor_scalar_mul(out_sb[:], po[:], inv[:])
            # out rides the SWDGE ring: on the HWDGE rings the 8-context
            # completion chain would add a second sync wait (walrus limit).
            nc.gpsimd.dma_start(out=yout[:], in_=out_sb[:])

    return nc


def _get_nc():
    if "nc" not in _CACHE:
        _CACHE["nc"] = _build_nc()
    return _CACHE["nc"]


def _pack_inputs(x, W, b):
    x = np.asarray(x, dtype=np.float32)
    W = np.asarray(W, dtype=np.float32)
    b = np.asarray(b, dtype=np.float32)
    f8 = ml_dtypes.float8_e4m3

    aug = np.zeros((B, NPAD, D), dtype=f8)
    aug[:, :N, :] = x.astype(f8)
    # row n = chunk i*128 + partition p -> [B, p, i, D], then regroup into
    # DMA tiles so each dma_start reads one fully contiguous DRAM extent:
    # [B][tile][p][npr*128]
    augT = aug.reshape(B, NCH, P, D).transpose(0, 2, 1, 3)  # [B,P,NCH,D]
    parts = []
    for (p0, npr) in X_TILES:
        blk = augT[:, :, 2 * p0 : 2 * (p0 + npr), :].reshape(B, P, npr * P)
        parts.append(blk.reshape(B, P * npr * P))
    xcat = np.ascontiguousarray(np.concatenate(parts, axis=1))

    # Symmetry fold: drop the W rows for cov entries (d>=32, e<32) and add
    # them onto their mirrors (d<32, e>=32); pack the kept 3072 rows in the
    # same chunk/partition layout the device writes feat in.
    H = D // 2
    Wm = W.reshape(D, D, OUT)
    W1 = Wm[0:H].copy()                        # rows d<32, full e
    W1[:, H:, :] += Wm[H:, 0:H, :].transpose(1, 0, 2)
    W2 = Wm[H:, H:, :].reshape(8, 4 * H, OUT)  # BR block, 4 rows per chunk
    Wc = np.concatenate([W1.reshape(16, P, OUT), W2], axis=0)
    wp = np.ascontiguousarray(Wc.transpose(1, 0, 2)).reshape(
        P, KCF * OUT
    ).astype(np.float16)
    bias_new = SC_FEAT * (b + W[(D + 1) * np.arange(D)].sum(axis=0))
    bp = np.concatenate([bias_new, np.ones(BPC, np.float32)]).astype(
        np.float16
    ).reshape(1, OUT + BPC)

    # Host-exact mean correction + identity shift, in feat units:
    # rim[b] = 32*(s s^T/(N(N-1)) + I), laid out [D, BPC, D] per core.
    s = x.sum(axis=1, dtype=np.float64)  # [B, D]
    rims = SC_FEAT * (
        np.einsum("bd,be->bde", s, s) / (N * (N - 1.0))
        + np.eye(D, dtype=np.float64)[None]
    ).astype(np.float32)  # [B, D, D]

    return [
        {
            "xin": np.ascontiguousarray(xcat[c * BPC : (c + 1) * BPC]),
            "win": wp,
            "bin": bp,
            "rim": np.ascontiguousarray(
                rims[c * BPC : (c + 1) * BPC].transpose(1, 0, 2)
            ).reshape(D, BPC * D),
        }
        for c in range(NCORES)
    ]


def run(x, W, b, trace=False):
    from concourse.bass_utils import run_bass_kernel_spmd

    nc = _get_nc()
    in_maps = _pack_inputs(x, W, b)
    res = run_bass_kernel_spmd(nc, in_maps, list(range(NCORES)), trace=trace)
    out = np.concatenate(
        [res.results[c]["yout"] for c in range(NCORES)], axis=0
    ).astype(np.float32)
    return out, res


def kernel(x, W, b):
    out, _ = run(x, W, b, trace=False)
    return out
